# revision 4
# baseline (speedup 1.0000x reference)
"""Trainium2 Bass kernel for BasicPropertiesRCNNHead (fc1+relu -> fc2+relu -> sigmoid -> top-3).

Contract: kernel(**inputs) takes FULL unsharded inputs (as produced by the
problem's setup_inputs) and returns the full (prob, top_idx) outputs.

Strategy:
- Data-parallel over N=16384 rows: 2048 rows per each of the 8 NeuronCores.
- Host pre-transposes x (to [D, n]) and splits x / fc1_w into fp16 hi+lo
  halves; fc1 is computed as 3 accumulating fp16 matmuls
  (xh*wh + xl*wh + xh*wl), which carries ~22-bit effective input precision
  (error is dominated by fp32 PSUM accumulation, same as a native fp32
  matmul) at 3/4 the PE cost of fp32.
- fc2 (288x288) runs in fp32 via small PE transposes of h.
- Top-3 per row is computed from the fc2 logits (monotone equivalent of
  sigmoid probs) with the DVE max/max_index instructions; entries with
  logit <= 0 (prob <= 0.5) fall back to index 0, matching the reference's
  thresholded top-k.
"""

import sys

if "/opt/trn_rl_repo" not in sys.path:
    sys.path.insert(0, "/opt/trn_rl_repo")

import numpy as np

N_TOTAL = 16384
D = 12544
NCLS = 288
NCORES = 8
PER = N_TOTAL // NCORES      # 2048 rows per core
KC = D // 128                # 98 contraction chunks of 128
QN = 512                     # rows handled per "quarter"
NQ = PER // QN               # 4 quarters
MQ = QN // 128               # 4 psum row-tiles per quarter
KG = 14                      # k-chunks per x DMA group
NGK = KC // KG               # 7 groups
NT = PER // 128              # 16 row-tiles per core

_CACHE = {}


def _build_program():
    import concourse.mybir as mybir
    from concourse import bacc
    from concourse.tile import TileContext
    from concourse.masks import make_identity

    F32 = mybir.dt.float32
    F16 = mybir.dt.float16
    U32 = mybir.dt.uint32
    nc = bacc.Bacc("TRN2", target_bir_lowering=False, debug=False)

    xh_h = nc.declare_dram_parameter("xh", [D, PER], F16, False)
    xl_h = nc.declare_dram_parameter("xl", [D, PER], F16, False)
    w1h_h = nc.declare_dram_parameter("w1h", [D, NCLS], F16, False)
    w1l_h = nc.declare_dram_parameter("w1l", [D, NCLS], F16, False)
    w2t_h = nc.declare_dram_parameter("w2t", [NCLS, NCLS], F32, False)
    b1_h = nc.declare_dram_parameter("b1", [NCLS], F32, False)
    b2_h = nc.declare_dram_parameter("b2", [NCLS], F32, False)
    prob_h = nc.declare_dram_parameter("prob", [PER, NCLS], F32, True)
    idx_h = nc.declare_dram_parameter("idx", [PER, 3], U32, True)

    with TileContext(nc) as tc:
        with (
            tc.tile_pool(name="wpool", bufs=1) as wpool,
            tc.tile_pool(name="xpool", bufs=2) as xpool,
            tc.tile_pool(name="tpool", bufs=3) as tpool,
            tc.tile_pool(name="hps_p", bufs=4, space="PSUM") as hps_pool,
            tc.tile_pool(name="zps_p", bufs=2, space="PSUM") as zps_pool,
            tc.tile_pool(name="tps_p", bufs=2, space="PSUM") as tps_pool,
        ):
            # ---- resident weights / constants ----
            # w1 halves as [128, KC*NCLS] (chunk k at cols k*NCLS..)
            w1h_sb = wpool.tile([128, KC * NCLS], F16, tag="w1h")
            w1l_sb = wpool.tile([128, KC * NCLS], F16, tag="w1l")
            for t, h in ((w1h_sb, w1h_h), (w1l_sb, w1l_h)):
                for part in range(2):  # two DMAs of 49 chunks each
                    k0 = part * (KC // 2)
                    ksz = KC // 2
                    nc.sync.dma_start(
                        out=t[:, k0 * NCLS:(k0 + ksz) * NCLS].rearrange(
                            "p (k c) -> p k c", k=ksz),
                        in_=h[k0 * 128:(k0 + ksz) * 128, :].rearrange(
                            "(k p) c -> p k c", p=128),
                    )
            # w2t as [96, 3*NCLS] (c-chunk j at cols j*NCLS..)
            w2t_sb = wpool.tile([96, 3 * NCLS], F32, tag="w2t")
            nc.sync.dma_start(
                out=w2t_sb.rearrange("p (j k) -> p j k", j=3),
                in_=w2t_h[:, :].rearrange("(j p) k -> p j k", p=96),
            )
            b1t = wpool.tile([128, NCLS], F32, tag="b1t")
            nc.gpsimd.dma_start(out=b1t, in_=b1_h[:].unsqueeze(0).broadcast_to([128, NCLS]))
            b2t = wpool.tile([128, NCLS], F32, tag="b2t")
            nc.gpsimd.dma_start(out=b2t, in_=b2_h[:].unsqueeze(0).broadcast_to([128, NCLS]))
            ident = wpool.tile([128, 128], F32, tag="ident")
            make_identity(nc, ident)
            zeros3 = wpool.tile([128, 3], U32, tag="zeros3")
            nc.gpsimd.memset(zeros3, 0)
            idx_acc = wpool.tile([128, NT * 3], U32, tag="idx_acc")

            for q in range(NQ):
                hps = [hps_pool.tile([128, NCLS], F32, tag="hps", name=f"hps_q{q}m{m}")
                       for m in range(MQ)]
                n0 = q * QN
                for g in range(NGK):
                    xh_g = xpool.tile([128, KG * QN], F16, tag="xh_g")
                    xl_g = xpool.tile([128, KG * QN], F16, tag="xl_g")
                    for t, h in ((xh_g, xh_h), (xl_g, xl_h)):
                        nc.sync.dma_start(
                            out=t.rearrange("p (k n) -> p k n", k=KG),
                            in_=h[g * KG * 128:(g + 1) * KG * 128,
                                  n0:n0 + QN].rearrange("(k p) n -> p k n", p=128),
                        )
                    for m in range(MQ):
                        for j in range(KG):
                            k = g * KG + j
                            xh_k = xh_g[:, j * QN + m * 128: j * QN + (m + 1) * 128]
                            xl_k = xl_g[:, j * QN + m * 128: j * QN + (m + 1) * 128]
                            w1h_k = w1h_sb[:, k * NCLS:(k + 1) * NCLS]
                            w1l_k = w1l_sb[:, k * NCLS:(k + 1) * NCLS]
                            first = (k == 0)
                            last = (k == KC - 1)
                            nc.tensor.matmul(hps[m], lhsT=xh_k, rhs=w1h_k,
                                             start=first, stop=False)
                            nc.tensor.matmul(hps[m], lhsT=xl_k, rhs=w1h_k,
                                             start=False, stop=False)
                            nc.tensor.matmul(hps[m], lhsT=xh_k, rhs=w1l_k,
                                             start=False, stop=last)
                # tail for this quarter's 4 row-tiles
                for m in range(MQ):
                    t_glob = q * MQ + m
                    h_sb = tpool.tile([128, NCLS], F32, tag="h_sb")
                    nc.vector.tensor_tensor(out=h_sb, in0=hps[m], in1=b1t,
                                            op=mybir.AluOpType.add)
                    nc.vector.tensor_scalar_max(h_sb, h_sb, 0.0)
                    # hT via 3 PE transposes of [128, 96] -> [96, 128]
                    hT_ps = tps_pool.tile([96, 3 * 128], F32, tag="hT_ps")
                    for j in range(3):
                        nc.tensor.transpose(
                            hT_ps[:, j * 128:(j + 1) * 128],
                            h_sb[:, j * 96:(j + 1) * 96], ident)
                    hT_sb = tpool.tile([96, 3 * 128], F32, tag="hT_sb")
                    nc.vector.tensor_copy(hT_sb, hT_ps)
                    # fc2: z = hT.T @ w2t (3 accumulating chunks of K=96)
                    zps = zps_pool.tile([128, NCLS], F32, tag="zps")
                    for j in range(3):
                        nc.tensor.matmul(
                            zps,
                            lhsT=hT_sb[:, j * 128:(j + 1) * 128],
                            rhs=w2t_sb[:, j * NCLS:(j + 1) * NCLS],
                            start=(j == 0), stop=(j == 2))
                    # z += b2 (in psum), prob = max(sigmoid(z), 0.5)
                    nc.vector.tensor_tensor(out=zps, in0=zps, in1=b2t,
                                            op=mybir.AluOpType.add)
                    prob_sb = tpool.tile([128, NCLS], F32, tag="prob_sb")
                    nc.scalar.activation(prob_sb, zps,
                                         mybir.ActivationFunctionType.Sigmoid)
                    nc.vector.tensor_scalar_max(prob_sb, prob_sb, 0.5)
                    nc.sync.dma_start(
                        out=prob_h[t_glob * 128:(t_glob + 1) * 128, :],
                        in_=prob_sb)
                    # top-3 on logits (monotone equiv of prob); <=0 -> idx 0
                    mx = tpool.tile([128, 8], F32, tag="mx")
                    nc.vector.max(out=mx, in_=zps)
                    mi = tpool.tile([128, 8], U32, tag="mi")
                    nc.vector.max_index(out=mi, in_max=mx, in_values=zps)
                    le_mask = tpool.tile([128, 3], U32, tag="le_mask")
                    nc.vector.tensor_scalar(
                        out=le_mask, in0=mx[:, 0:3], scalar1=0.0, scalar2=None,
                        op0=mybir.AluOpType.is_le)
                    nc.vector.copy_predicated(out=mi[:, 0:3], mask=le_mask,
                                              data=zeros3)
                    # reversed (ascending-score) order into the accumulator
                    for j in range(3):
                        nc.vector.tensor_copy(
                            idx_acc[:, t_glob * 3 + j: t_glob * 3 + j + 1],
                            mi[:, 2 - j: 3 - j])
            nc.sync.dma_start(
                out=idx_h[:, :].rearrange("(t p) j -> p t j", p=128),
                in_=idx_acc.rearrange("p (t j) -> p t j", t=NT),
            )

    nc.compile()
    return nc


def _get_program():
    if "nc" not in _CACHE:
        _CACHE["nc"] = _build_program()
    return _CACHE["nc"]


def kernel(x, fc1_w, fc1_b, fc2_w, fc2_b, _trace=False, _trace_dir=None):
    from concourse.bass_utils import run_bass_kernel_spmd

    x = np.asarray(x, dtype=np.float32).reshape(N_TOTAL, D)
    fc1_w = np.asarray(fc1_w, dtype=np.float32)
    fc1_b = np.asarray(fc1_b, dtype=np.float32)
    fc2_w = np.asarray(fc2_w, dtype=np.float32)
    fc2_b = np.asarray(fc2_b, dtype=np.float32)

    w1t = np.ascontiguousarray(fc1_w.T)            # [D, NCLS]
    w1h = w1t.astype(np.float16)
    w1l = (w1t - w1h.astype(np.float32)).astype(np.float16)
    w2t = np.ascontiguousarray(fc2_w.T)            # [NCLS, NCLS] (c, k)

    nc = _get_program()

    in_maps = []
    for c in range(NCORES):
        shard = np.ascontiguousarray(x[c * PER:(c + 1) * PER].T)  # [D, PER]
        xh = shard.astype(np.float16)
        xl = (shard - xh.astype(np.float32)).astype(np.float16)
        in_maps.append(dict(xh=xh, xl=xl, w1h=w1h, w1l=w1l, w2t=w2t,
                            b1=fc1_b, b2=fc2_b))

    res = run_bass_kernel_spmd(nc, in_maps, list(range(NCORES)),
                               trace=_trace, tmpdir=_trace_dir)
    _CACHE["last_res"] = res
    prob = np.concatenate([r["prob"] for r in res.results], axis=0)
    idx = np.concatenate([r["idx"] for r in res.results], axis=0).view(np.int32)
    return prob, idx


# revision 6
# speedup vs baseline: 1.0569x; 1.0569x over previous
"""Trainium2 Bass kernel for BasicPropertiesRCNNHead (fc1+relu -> fc2+relu -> sigmoid -> top-3).

Contract: kernel(**inputs) takes FULL unsharded inputs (as produced by the
problem's setup_inputs) and returns the full (prob, top_idx) outputs.

Strategy:
- Data-parallel over N=16384 rows: 2048 rows per each of the 8 NeuronCores.
- Host pre-transposes x (to [D, n]) and splits x / fc1_w into fp16 hi+lo
  halves; fc1 is computed as 3 accumulating fp16 matmuls
  (xh*wh + xl*wh + xh*wl), which carries ~22-bit effective input precision
  (error is dominated by fp32 PSUM accumulation, same as a native fp32
  matmul) at 3/4 the PE cost of fp32.
- fc2 (288x288) runs in fp32 via small PE transposes of h.
- Top-3 per row is computed from the fc2 logits (monotone equivalent of
  sigmoid probs) with the DVE max/max_index instructions; entries with
  logit <= 0 (prob <= 0.5) fall back to index 0, matching the reference's
  thresholded top-k.
"""

import sys

if "/opt/trn_rl_repo" not in sys.path:
    sys.path.insert(0, "/opt/trn_rl_repo")

import numpy as np

N_TOTAL = 16384
D = 12544
NCLS = 288
NCORES = 8
PER = N_TOTAL // NCORES      # 2048 rows per core
KC = D // 128                # 98 contraction chunks of 128
QN = 512                     # rows handled per "quarter"
NQ = PER // QN               # 4 quarters
MQ = QN // 128               # 4 psum row-tiles per quarter
KG = 14                      # k-chunks per x DMA group
NGK = KC // KG               # 7 groups
NT = PER // 128              # 16 row-tiles per core

_CACHE = {}


def _build_program():
    import concourse.mybir as mybir
    from concourse import bacc
    from concourse.tile import TileContext
    from concourse.masks import make_identity

    F32 = mybir.dt.float32
    F16 = mybir.dt.float16
    U32 = mybir.dt.uint32
    nc = bacc.Bacc("TRN2", target_bir_lowering=False, debug=False)

    xh_h = nc.declare_dram_parameter("xh", [D, PER], F16, False)
    xl_h = nc.declare_dram_parameter("xl", [D, PER], F16, False)
    w1h_h = nc.declare_dram_parameter("w1h", [D, NCLS], F16, False)
    w1l_h = nc.declare_dram_parameter("w1l", [D, NCLS], F16, False)
    w2t_h = nc.declare_dram_parameter("w2t", [NCLS, NCLS], F32, False)
    b1_h = nc.declare_dram_parameter("b1", [NCLS], F32, False)
    b2_h = nc.declare_dram_parameter("b2", [NCLS], F32, False)
    prob_h = nc.declare_dram_parameter("prob", [PER, NCLS], F32, True)
    idx_h = nc.declare_dram_parameter("idx", [PER, 3], U32, True)

    with TileContext(nc) as tc:
        with (
            tc.tile_pool(name="wpool", bufs=1) as wpool,
            tc.tile_pool(name="xpool", bufs=2) as xpool,
            tc.tile_pool(name="tpool", bufs=3) as tpool,
            tc.tile_pool(name="hps_p", bufs=4, space="PSUM") as hps_pool,
            tc.tile_pool(name="zps_p", bufs=2, space="PSUM") as zps_pool,
            tc.tile_pool(name="tps_p", bufs=2, space="PSUM") as tps_pool,
        ):
            # ---- resident weights / constants ----
            # w1 halves in KG-chunk group tiles; hi/lo groups interleaved on
            # the sync HWDGE ring so the first matmul only waits ~4 MB.
            w1h_g = []
            w1l_g = []
            for g in range(NGK):
                th = wpool.tile([128, KG * NCLS], F16, tag=f"w1h{g}",
                                name=f"w1h_g{g}")
                tl = wpool.tile([128, KG * NCLS], F16, tag=f"w1l{g}",
                                name=f"w1l_g{g}")
                for t, h in ((th, w1h_h), (tl, w1l_h)):
                    nc.sync.dma_start(
                        out=t.rearrange("p (k c) -> p k c", k=KG),
                        in_=h[g * KG * 128:(g + 1) * KG * 128, :].rearrange(
                            "(k p) c -> p k c", p=128),
                    )
                w1h_g.append(th)
                w1l_g.append(tl)
            # w2t as [96, 3*NCLS] (c-chunk j at cols j*NCLS..); gpsimd ring
            # (only needed once the first tail runs)
            w2t_sb = wpool.tile([96, 3 * NCLS], F32, tag="w2t")
            nc.gpsimd.dma_start(
                out=w2t_sb.rearrange("p (j k) -> p j k", j=3),
                in_=w2t_h[:, :].rearrange("(j p) k -> p j k", p=96),
            )
            b1t = wpool.tile([128, NCLS], F32, tag="b1t")
            nc.gpsimd.dma_start(out=b1t, in_=b1_h[:].unsqueeze(0).broadcast_to([128, NCLS]))
            b2t = wpool.tile([128, NCLS], F32, tag="b2t")
            nc.gpsimd.dma_start(out=b2t, in_=b2_h[:].unsqueeze(0).broadcast_to([128, NCLS]))
            ident = wpool.tile([128, 128], F32, tag="ident")
            make_identity(nc, ident)
            zeros3 = wpool.tile([128, 3], U32, tag="zeros3")
            nc.gpsimd.memset(zeros3, 0)
            idx_acc = wpool.tile([128, NT * 3], U32, tag="idx_acc")

            for q in range(NQ):
                hps = [hps_pool.tile([128, NCLS], F32, tag="hps", name=f"hps_q{q}m{m}")
                       for m in range(MQ)]
                n0 = q * QN
                for g in range(NGK):
                    xh_g = xpool.tile([128, KG * QN], F16, tag="xh_g")
                    xl_g = xpool.tile([128, KG * QN], F16, tag="xl_g")
                    for t, h in ((xh_g, xh_h), (xl_g, xl_h)):
                        nc.scalar.dma_start(
                            out=t.rearrange("p (k n) -> p k n", k=KG),
                            in_=h[g * KG * 128:(g + 1) * KG * 128,
                                  n0:n0 + QN].rearrange("(k p) n -> p k n", p=128),
                        )
                    for m in range(MQ):
                        for j in range(KG):
                            k = g * KG + j
                            xh_k = xh_g[:, j * QN + m * 128: j * QN + (m + 1) * 128]
                            xl_k = xl_g[:, j * QN + m * 128: j * QN + (m + 1) * 128]
                            w1h_k = w1h_g[g][:, j * NCLS:(j + 1) * NCLS]
                            w1l_k = w1l_g[g][:, j * NCLS:(j + 1) * NCLS]
                            first = (k == 0)
                            last = (k == KC - 1)
                            nc.tensor.matmul(hps[m], lhsT=xh_k, rhs=w1h_k,
                                             start=first, stop=False)
                            nc.tensor.matmul(hps[m], lhsT=xl_k, rhs=w1h_k,
                                             start=False, stop=False)
                            nc.tensor.matmul(hps[m], lhsT=xh_k, rhs=w1l_k,
                                             start=False, stop=last)
                # tail for this quarter's 4 row-tiles
                for m in range(MQ):
                    t_glob = q * MQ + m
                    h_sb = tpool.tile([128, NCLS], F32, tag="h_sb")
                    nc.vector.tensor_tensor(out=h_sb, in0=hps[m], in1=b1t,
                                            op=mybir.AluOpType.add)
                    nc.vector.tensor_scalar_max(h_sb, h_sb, 0.0)
                    # hT via 3 PE transposes of [128, 96] -> [96, 128]
                    hT_ps = tps_pool.tile([96, 3 * 128], F32, tag="hT_ps")
                    for j in range(3):
                        nc.tensor.transpose(
                            hT_ps[:, j * 128:(j + 1) * 128],
                            h_sb[:, j * 96:(j + 1) * 96], ident)
                    hT_sb = tpool.tile([96, 3 * 128], F32, tag="hT_sb")
                    nc.vector.tensor_copy(hT_sb, hT_ps)
                    # fc2: z = hT.T @ w2t (3 accumulating chunks of K=96)
                    zps = zps_pool.tile([128, NCLS], F32, tag="zps")
                    for j in range(3):
                        nc.tensor.matmul(
                            zps,
                            lhsT=hT_sb[:, j * 128:(j + 1) * 128],
                            rhs=w2t_sb[:, j * NCLS:(j + 1) * NCLS],
                            start=(j == 0), stop=(j == 2))
                    # z += b2 (in psum), prob = max(sigmoid(z), 0.5)
                    nc.vector.tensor_tensor(out=zps, in0=zps, in1=b2t,
                                            op=mybir.AluOpType.add)
                    prob_sb = tpool.tile([128, NCLS], F32, tag="prob_sb")
                    nc.scalar.activation(prob_sb, zps,
                                         mybir.ActivationFunctionType.Sigmoid)
                    nc.vector.tensor_scalar_max(prob_sb, prob_sb, 0.5)
                    nc.sync.dma_start(
                        out=prob_h[t_glob * 128:(t_glob + 1) * 128, :],
                        in_=prob_sb)
                    # top-3 on logits (monotone equiv of prob); <=0 -> idx 0
                    mx = tpool.tile([128, 8], F32, tag="mx")
                    nc.vector.max(out=mx, in_=zps)
                    mi = tpool.tile([128, 8], U32, tag="mi")
                    nc.vector.max_index(out=mi, in_max=mx, in_values=zps)
                    le_mask = tpool.tile([128, 3], U32, tag="le_mask")
                    nc.vector.tensor_scalar(
                        out=le_mask, in0=mx[:, 0:3], scalar1=0.0, scalar2=None,
                        op0=mybir.AluOpType.is_le)
                    nc.vector.copy_predicated(out=mi[:, 0:3], mask=le_mask,
                                              data=zeros3)
                    # reversed (ascending-score) order into the accumulator
                    for j in range(3):
                        nc.vector.tensor_copy(
                            idx_acc[:, t_glob * 3 + j: t_glob * 3 + j + 1],
                            mi[:, 2 - j: 3 - j])
            nc.sync.dma_start(
                out=idx_h[:, :].rearrange("(t p) j -> p t j", p=128),
                in_=idx_acc.rearrange("p (t j) -> p t j", t=NT),
            )

    nc.compile()
    return nc


def _get_program():
    if "nc" not in _CACHE:
        _CACHE["nc"] = _build_program()
    return _CACHE["nc"]


def kernel(x, fc1_w, fc1_b, fc2_w, fc2_b, _trace=False, _trace_dir=None):
    from concourse.bass_utils import run_bass_kernel_spmd

    x = np.asarray(x, dtype=np.float32).reshape(N_TOTAL, D)
    fc1_w = np.asarray(fc1_w, dtype=np.float32)
    fc1_b = np.asarray(fc1_b, dtype=np.float32)
    fc2_w = np.asarray(fc2_w, dtype=np.float32)
    fc2_b = np.asarray(fc2_b, dtype=np.float32)

    w1t = np.ascontiguousarray(fc1_w.T)            # [D, NCLS]
    w1h = w1t.astype(np.float16)
    w1l = (w1t - w1h.astype(np.float32)).astype(np.float16)
    w2t = np.ascontiguousarray(fc2_w.T)            # [NCLS, NCLS] (c, k)

    nc = _get_program()

    in_maps = []
    for c in range(NCORES):
        shard = np.ascontiguousarray(x[c * PER:(c + 1) * PER].T)  # [D, PER]
        xh = shard.astype(np.float16)
        xl = (shard - xh.astype(np.float32)).astype(np.float16)
        in_maps.append(dict(xh=xh, xl=xl, w1h=w1h, w1l=w1l, w2t=w2t,
                            b1=fc1_b, b2=fc2_b))

    res = run_bass_kernel_spmd(nc, in_maps, list(range(NCORES)),
                               trace=_trace, tmpdir=_trace_dir)
    _CACHE["last_res"] = res
    prob = np.concatenate([r["prob"] for r in res.results], axis=0)
    idx = np.concatenate([r["idx"] for r in res.results], axis=0).view(np.int32)
    return prob, idx
